# revision 14
# baseline (speedup 1.0000x reference)
"""GAT single-head forward on 8 Trainium2 NeuronCores (Bass/Tile).

Math (per reference):
    h   = X @ W + b                      [N, 128]
    f1  = h @ v0, f2 = h @ v1            [N]
    logits = adj * (f1[:,None] + f2[None,:])   (adj entries are exactly 0/1)
    vals = sigmoid(logits) - 0.5
    masked softmax over row edges; out = probs @ h

Key identities used on device:
  * On edges: softmax weight w = exp(sigmoid(s)), s = f1_i + f2_j; with
    t = tanh(s/2): w = exp((1+t)/2) up to a global factor that cancels in
    the row-normalization.
  * QUAD: exp((1+t)/2) ~ rho*[(t+d)^2 + C] on t in [-1,1] (minimax rel err
    5.4e-3; end-to-end 3.7e-3 incl fp16). This removes the dense EXP pass:
    the only dense activation is ONE tanh.
  * Mask folding: m = (f1/2 + f2/2)*A makes masked entries exactly 0, so
    tanh gives t=0 there. The DVE-route quad P=(t+2d)*t is then exactly 0
    on non-edges; the ACT-route quad P=Square(t+d) gives the constant
    fl16(d^2) there, corrected by subtracting kS*S_tot (S_tot = column
    sums of h_ext) in the epilogue. The "+C" of the quad is realized as a
    second aggregate matmul A @ (C+d^2)*h_ext accumulated into a separate
    PSUM region (the PE has slack; ACT/DVE do not).
  * The quadratic itself is computed by BOTH engines, split by i-columns:
    ACT Square(t+d) for i < NSQ, DVE scalar_tensor_tensor (t+2d)*t for the
    rest -- algebraically the same function, balancing ACT vs DVE load.
  * A ones-column appended to h turns the softmax denominator into one
    extra matmul output column.

Sharding: rows of adj across the 8 cores (1024 rows each). node_feats is
replicated; every core computes the full projected h - no collectives.

Per-core layout: adj block TRANSPOSED ([j=source node on partitions,
i=own rows on free dim]) so the aggregates contract over the partition
dim. adj is cast to fp16 host-side (exact for a 0/1 mask).

Dense-pass engine budget per core (8.4M elements each):
  DVE: stt1 mask-mul (per-q) + stt2 quad on (1024-NSQ)/1024 of columns
  ACT: tanh (fused per group) + Square on NSQ/1024 of columns
  PE : P@h_ext + A@h_ext (1024 matmuls of 129 cols) + h-projection + S_tot
"""

import os

import numpy as np

import concourse.mybir as mybir
import concourse.tile as tile
from concourse import bacc
from concourse.bass_utils import run_bass_kernel_spmd

F32 = mybir.dt.float32
F16 = mybir.dt.float16
AF = mybir.ActivationFunctionType
ALU = mybir.AluOpType

N, C_IN, C_OUT = 8192, 256, 128
NCORES = 8
ROWS = N // NCORES          # 1024 rows of adj per core
P = 128
NT = N // P                 # 64 node tiles (j-chunks)
NI = ROWS // P              # 8 output row-tiles per core
KC = [128, 128, 1]          # contraction chunks of K=257
WCOLS = C_OUT + 3           # [W | ones-hack | 0.5*w0 | 0.5*w1]
HCOLS = C_OUT + 1           # h plus the ones column
TINY = float(np.finfo(np.float32).tiny)
BANK = 512                  # PSUM bank, fp32 elements

# quadratic approx of exp((1+t)/2) ~ rho*[(t+D)^2 + CC]  (rho cancels)
D = 2.079251
CC = 3.749643
CP = float(np.float16(CC + D * D))    # adj host-prescale = A-aggregate coef
KS = float(np.float16(np.float32(D) ** 2))  # masked ACT-route P value

# j-chunk split: within each fusion group the first NS_OF(gsz) chunks run
# the ACT Square route, the rest the DVE stt route (flat contiguous column
# ranges -- multi-block strided write APs have unreliable dependency
# tracking). Tunable ACT/DVE balance knob.
SQX = 0.5

def NS_OF(gsz):
    return int(round(gsz * SQX))

GROUPS = [2, 4] + [6] * 9 + [4]       # j-chunk fusion per ACT/DVE instr

_CACHE: dict = {}


def _build_nc(b_zero=True, debug_out=False):
    nc = bacc.Bacc(
        "TRN2", target_bir_lowering=False, debug=False, num_devices=NCORES
    )
    xt1 = nc.dram_tensor("xt1", [257, N], F16, kind="ExternalInput").ap()
    xt1l = nc.dram_tensor("xt1l", [257, ROWS], F16, kind="ExternalInput").ap()
    wext = nc.dram_tensor("wext", [257, WCOLS], F16, kind="ExternalInput").ap()
    adjt = nc.dram_tensor("adjt", [N, ROWS], F16, kind="ExternalInput").ap()
    out = nc.dram_tensor("out", [ROWS, C_OUT], F32, kind="ExternalOutput").ap()
    dbg = None
    if debug_out:
        dbg = {
            "pq": nc.dram_tensor("dbg_pq", [P, 8 * ROWS], F16, kind="ExternalOutput").ap(),
            "mt": nc.dram_tensor("dbg_mt", [P, 8 * ROWS], F16, kind="ExternalOutput").ap(),
            "ssb": nc.dram_tensor("dbg_ssb", [P, HCOLS], F32, kind="ExternalOutput").ap(),
            "ns": nc.dram_tensor("dbg_ns", [P, NI * HCOLS], F32, kind="ExternalOutput").ap(),
        }

    with tile.TileContext(nc) as tc:
        _emit(tc, nc, xt1, xt1l, wext, adjt, out, b_zero, dbg)
    nc.compile()
    return nc


def _emit(tc, nc, xt1, xt1l, wext, adjt, out, b_zero, dbg=None):
    from contextlib import ExitStack

    nkc = 2 if b_zero else 3

    with ExitStack() as ctx:
        # ---- persistent tiles ----
        persist = ctx.enter_context(tc.tile_pool(name="persist", bufs=1))
        h16_all = persist.tile([P, NT * HCOLS], F16, tag="h16")   # [128, 8256]
        f2h_all = persist.tile([P, NT], F32, tag="f2h")           # 0.5*f2 per j
        f1rep = persist.tile([P, ROWS], F16, tag="f1rep")         # 0.5*f1 bcast
        ones1 = persist.tile([P, 1], F16, tag="ones1")
        nc.vector.memset(ones1[:], 1.0)
        zero1 = persist.tile([P, 1], F32, tag="zero1")
        nc.vector.memset(zero1[:], 0.0)
        dbias = persist.tile([P, 1], F32, tag="dbias")
        nc.vector.memset(dbias[:], float(D))
        if b_zero:
            nc.vector.memset(
                h16_all[:].rearrange("p (t c) -> p t c", c=HCOLS)[
                    :, :, C_OUT : C_OUT + 1
                ],
                1.0,
            )

        xtp = ctx.enter_context(tc.tile_pool(name="xt", bufs=1))

        # ---- input loads (same staging as before) ----
        offs = [0, 128, 256]
        xts = [
            xtp.tile([KC[k], N], F16, name=f"xtsb{k}", tag=f"xt{k}")
            for k in range(nkc)
        ]
        SUBS = [0, 1024, 3072, 5120, N]
        wes, xls = [], []
        off = 0
        for k in range(nkc):
            kc = KC[k]
            wx_sb = xtp.tile([kc, WCOLS + ROWS], F16, name=f"wx{k}", tag=f"wx{k}")
            nc.sync.dma_start(wx_sb[:, 0:WCOLS], wext[off : off + kc, :])
            nc.sync.dma_start(wx_sb[:, WCOLS:], xt1l[off : off + kc, :])
            wes.append(wx_sb[:, 0:WCOLS])
            xls.append(wx_sb[:, WCOLS:])
            off += kc
        for k in range(nkc):
            if KC[k] == P:
                nc.sync.dma_start(
                    xts[k][:, 0 : SUBS[1]],
                    xt1[offs[k] : offs[k] + KC[k], 0 : SUBS[1]],
                )
        for c in range(1, len(SUBS) - 1):
            for k in range(nkc):
                if KC[k] != P:
                    if c == 1:
                        nc.sync.dma_start(
                            xts[k][:], xt1[offs[k] : offs[k] + KC[k], :]
                        )
                    continue
                nc.sync.dma_start(
                    xts[k][:, SUBS[c] : SUBS[c + 1]],
                    xt1[offs[k] : offs[k] + KC[k], SUBS[c] : SUBS[c + 1]],
                )

        # ---- f1 path: 0.5*f1 replicated across partitions via matmul with
        # the 0.5*w0 column broadcast across PE columns ----
        with tc.tile_pool(name="pf", bufs=1, space="PSUM") as pfp:
            prep = pfp.tile([P, ROWS], F32, tag="prep")
            for k in range(nkc):
                for nh in range(ROWS // 512):
                    nc.tensor.matmul(
                        prep[:, nh * 512 : (nh + 1) * 512],
                        wes[k][:, C_OUT + 1 : C_OUT + 2].to_broadcast(
                            (KC[k], P)
                        ),
                        xls[k][:, nh * 512 : (nh + 1) * 512],
                        start=(k == 0),
                        stop=(k == nkc - 1),
                    )
            nc.scalar.copy(f1rep[:], prep[:])

        # ---- f2 head start: 0.5*f2 for the first 8 j-chunks ----
        F2HEAD = 8
        with tc.tile_pool(name="pf2", bufs=1, space="PSUM") as pf2p:
            pt = pf2p.tile([P, NI * BANK], F32, tag="pt")
            pt3 = pt[:].rearrange("p (t w) -> p t w", w=BANK)
            for q in range(F2HEAD):
                w = (q % NI) * BANK
                for k in range(nkc):
                    nc.tensor.matmul(
                        pt[:, w : w + 1],
                        xts[k][:, q * P : (q + 1) * P],
                        wes[k][:, C_OUT + 2 : C_OUT + 3],
                        start=(k == 0),
                        stop=(k == nkc - 1),
                    )
                if q == 1:
                    nc.vector.tensor_copy(
                        f2h_all[:, 0:2], pt3[:, 0:2, 0:1]
                    )
            nc.vector.tensor_copy(
                f2h_all[:, 2:F2HEAD], pt3[:, 2:F2HEAD, 0:1]
            )

        # ---- main-loop pools ----
        atp = ctx.enter_context(tc.tile_pool(name="atp", bufs=3))   # adj tiles
        mtp = ctx.enter_context(tc.tile_pool(name="mtp", bufs=3))   # m/t (in-place)
        ptp = ctx.enter_context(tc.tile_pool(name="ptp", bufs=3))   # quad P
        obp = ctx.enter_context(tc.tile_pool(name="ob", bufs=2))

        group_q0 = []
        q0 = 0
        for gsz in GROUPS:
            group_q0.append(q0)
            q0 += gsz

        deferred = []

        def emit_group_front(g):
            """adj DMA, per-q masked preadd (stt), fused tanh, quad split."""
            gsz = GROUPS[g]
            q0 = group_q0[g]
            at_sup = atp.tile([P, gsz * ROWS], F16, tag="at", name=f"at{g}")
            nc.sync.dma_start(
                at_sup[:].rearrange("p (q i) -> p q i", i=ROWS),
                adjt.rearrange("(q p) i -> p q i", p=P)[:, q0 : q0 + gsz, :],
            )
            mt = mtp.tile([P, gsz * ROWS], F16, tag="mt", name=f"mt{g}")
            for qq in range(gsz):
                q = q0 + qq
                # m = (0.5*f1_i + 0.5*f2_j) * A   [j on partitions, i free]
                nc.vector.scalar_tensor_tensor(
                    mt[:, qq * ROWS : (qq + 1) * ROWS],
                    f1rep[:],
                    f2h_all[:, q : q + 1],
                    at_sup[:, qq * ROWS : (qq + 1) * ROWS],
                    ALU.add,
                    ALU.mult,
                )
            # t = tanh(m) in place; masked entries -> tanh(0) = 0
            nc.scalar.activation(mt[:], mt[:], AF.Tanh, bias=zero1[:], scale=float(1.0 / CP))
            pq = ptp.tile([P, gsz * ROWS], F16, tag="pq", name=f"pq{g}")
            ns_ = NS_OF(gsz)
            if ns_ > 0:
                # ACT route (first ns_ chunks): P = (t + D)^2 ;
                # masked -> fl16(D^2) = KS (corrected via S_QS)
                nc.scalar.activation(
                    pq[:, 0 : ns_ * ROWS], mt[:, 0 : ns_ * ROWS], AF.Square,
                    bias=dbias[:],
                )
            if ns_ < gsz:
                # DVE route (rest): P = (t + 2D)*t ; masked -> 0
                nc.vector.scalar_tensor_tensor(
                    pq[:, ns_ * ROWS :],
                    mt[:, ns_ * ROWS :],
                    float(2 * D),
                    mt[:, ns_ * ROWS :],
                    ALU.add,
                    ALU.mult,
                )
            if dbg is not None and g == 2:
                nc.sync.dma_start(dbg["pq"][:, 0 : gsz * ROWS], pq[:])
                nc.sync.dma_start(dbg["mt"][:, 0 : gsz * ROWS], mt[:])
            return {"g": g, "gsz": gsz, "q0": q0, "at": at_sup, "pq": pq}

        def emit_group_back(fr, po_all, mid=None):
            """dual aggregate matmuls for a prepared group."""
            gsz, q0, at_sup, pq = fr["gsz"], fr["q0"], fr["at"], fr["pq"]
            for qq in range(gsz):
                if qq == min(2, gsz - 1) and mid is not None:
                    mid()
                q = q0 + qq
                rhs = h16_all[:, q * HCOLS : (q + 1) * HCOLS]
                # P-matmuls then A-matmuls accumulate into the SAME psum
                # region (adj is host-prescaled by CP so the A term needs no
                # separate scale): one accumulation group per bank, started
                # by P-q0 only. Consecutive matmuls hit different banks.
                for it in range(NI):
                    nc.tensor.matmul(
                        po_all[:, it * BANK : it * BANK + HCOLS],
                        pq[:, qq * ROWS + it * P : qq * ROWS + (it + 1) * P],
                        rhs,
                        start=(q == 0),
                        stop=False,
                    )
                for it in range(NI):
                    nc.tensor.matmul(
                        po_all[:, it * BANK : it * BANK + HCOLS],
                        at_sup[:, qq * ROWS + it * P : qq * ROWS + (it + 1) * P],
                        rhs,
                        start=False,
                        stop=(q == NT - 1),
                    )

        # ---- h-projection (unchanged): 8 PSUM banks, batches of 4 ----
        next_group = 0
        with tc.tile_pool(name="php", bufs=1, space="PSUM") as php:
            ph_all = php.tile([P, NI * BANK], F32, tag="ph")
            for b in range(NT // 4):
                for half in range(2):
                    nt0 = 4 * b + 2 * half
                    w0 = (nt0 % NI) * BANK
                    w1 = ((nt0 + 1) % NI) * BANK
                    for k in range(nkc):
                        nc.tensor.matmul(
                            ph_all[:, w0 : w0 + WCOLS],
                            xts[k][:, nt0 * P : (nt0 + 1) * P],
                            wes[k][:],
                            start=(k == 0),
                            stop=(k == nkc - 1),
                        )
                        nc.tensor.matmul(
                            ph_all[:, w1 : w1 + WCOLS],
                            xts[k][:, (nt0 + 1) * P : (nt0 + 2) * P],
                            wes[k][:],
                            start=(k == 0),
                            stop=(k == nkc - 1),
                        )
                bt = 4 * b
                wlo = (bt % NI) * BANK
                src = ph_all[:, wlo : wlo + 4 * BANK].rearrange(
                    "p (b w) -> p b w", b=4
                )
                dst_h = h16_all[:, bt * HCOLS : (bt + 4) * HCOLS].rearrange(
                    "p (b w) -> p b w", b=4
                )
                hc = C_OUT if b_zero else HCOLS
                nc.vector.tensor_copy(dst_h[:, :, 0:hc], src[:, :, 0:hc])
                if bt >= 8:
                    nc.vector.tensor_copy(
                        f2h_all[:, bt : bt + 4],
                        src[:, :, C_OUT + 2 : C_OUT + 3],
                    )
                while (
                    next_group < len(GROUPS)
                    and group_q0[next_group] + GROUPS[next_group] <= 4 * (b + 1)
                    and len(deferred) < 2
                ):
                    deferred.append(emit_group_front(next_group))
                    next_group += 1

        # ---- aggregate accumulators: bank i holds [P-accum | A-accum] for
        # i-tile i; S_tot (ones @ h_ext) rides spare cols of banks 0/1 ----
        pop = ctx.enter_context(tc.tile_pool(name="po", bufs=1, space="PSUM"))
        po_all = pop.tile([P, NI * BANK], F32, tag="poall")

        # S_QS burst: column sums of h_ext over the ACT-route j-chunks.
        # One broadcast-ones stationary, matmuls alternating between two
        # regions (banks 0/1) to avoid same-bank serialization.
        QS = []
        qq0 = 0
        for gsz in GROUPS:
            QS += list(range(qq0, qq0 + NS_OF(gsz)))
            qq0 += gsz
        if QS:
            sreg = [
                po_all[:, 2 * HCOLS + 4 : 3 * HCOLS + 4],
                po_all[:, BANK + 2 * HCOLS + 4 : BANK + 3 * HCOLS + 4],
            ]
            for i, q in enumerate(QS):
                nc.tensor.matmul(
                    sreg[i % 2],
                    ones1[:].to_broadcast((P, P)),
                    h16_all[:, q * HCOLS : (q + 1) * HCOLS],
                    start=(i < 2),
                    stop=(i >= len(QS) - 2),
                )

        for g in range(next_group, len(GROUPS)):
            emit_group_back(
                deferred.pop(0),
                po_all,
                mid=lambda g=g: deferred.append(emit_group_front(g)),
            )
        for fr in deferred:
            emit_group_back(fr, po_all)

        # ---- epilogue ----
        # num = psum_P + CP*psum_A  (- KS*S_tot on the ACT-route i-tiles)
        ns = obp.tile([P, NI * HCOLS], F32, tag="ns")
        ns3 = ns[:].rearrange("p (t c) -> p t c", c=HCOLS)
        if QS:
            # combine the two S_QS half-accumulators (<=1 PSUM read per op)
            s_sb = obp.tile([P, HCOLS], F32, tag="ssb")
            nc.scalar.copy(s_sb[:], sreg[0])
            nc.vector.scalar_tensor_tensor(
                s_sb[:], sreg[1], 1.0, s_sb[:], ALU.mult, ALU.add
            )
        # each ns range is written by exactly ONE instruction (chained
        # writes to the same range have unreliable ordering)
        dm = obp.tile([P, NI], F32, tag="dm")
        for it in range(NI):
            if QS:
                nc.vector.scalar_tensor_tensor(
                    ns3[:, it, :],
                    s_sb[:],
                    -KS,
                    po_all[:, it * BANK : it * BANK + HCOLS],
                    ALU.mult,
                    ALU.add,
                )
            elif it % 2 == 0:
                nc.vector.tensor_copy(
                    ns3[:, it, :], po_all[:, it * BANK : it * BANK + HCOLS]
                )
            else:
                nc.scalar.copy(
                    ns3[:, it, :], po_all[:, it * BANK : it * BANK + HCOLS]
                )
            # flat per-it den read (avoid a strided read racing the stt)
            nc.vector.tensor_scalar_max(
                dm[:, it : it + 1], ns3[:, it, C_OUT : C_OUT + 1], TINY
            )
        if dbg is not None:
            if QS:
                nc.sync.dma_start(dbg["ssb"], s_sb[:])
            nc.sync.dma_start(dbg["ns"], ns[:])
        rc = obp.tile([P, NI], F32, tag="rc")
        nc.vector.reciprocal(rc[:], dm[:])
        ob_all = obp.tile([P, NI * C_OUT], F32, tag="oball")
        for it in range(NI):
            if it % 2 == 0:
                nc.vector.tensor_scalar_mul(
                    ob_all[:, it * C_OUT : (it + 1) * C_OUT],
                    ns3[:, it, 0:C_OUT],
                    rc[:, it : it + 1],
                )
            else:
                nc.scalar.mul(
                    ob_all[:, it * C_OUT : (it + 1) * C_OUT],
                    ns3[:, it, 0:C_OUT],
                    rc[:, it : it + 1],
                )
        nc.sync.dma_start(
            out.rearrange("(t p) c -> p t c", p=P),
            ob_all[:].rearrange("p (t c) -> p t c", c=C_OUT),
        )


def _prep_inputs(node_feats, adj_matrix, W, b, v0, v1):
    X = np.ascontiguousarray(node_feats, dtype=np.float32)
    W = np.asarray(W, dtype=np.float32)
    b = np.asarray(b, dtype=np.float32)
    v0 = np.asarray(v0, dtype=np.float32)
    v1 = np.asarray(v1, dtype=np.float32)

    w0h = (0.5 * (W.astype(np.float64) @ v0.astype(np.float64))).astype(np.float32)
    w1h = (0.5 * (W.astype(np.float64) @ v1.astype(np.float64))).astype(np.float32)
    c0h = np.float32(0.5 * float(b.astype(np.float64) @ v0.astype(np.float64)))
    c1h = np.float32(0.5 * float(b.astype(np.float64) @ v1.astype(np.float64)))

    XT1 = np.empty((257, N), np.float32)
    XT1[:256] = X.T
    XT1[256] = 1.0

    WE = np.zeros((257, WCOLS), np.float32)
    WE[:256, :C_OUT] = W
    WE[256, :C_OUT] = b
    WE[256, C_OUT] = 1.0
    WE[:256, C_OUT + 1] = w0h
    WE[256, C_OUT + 1] = c0h
    WE[:256, C_OUT + 2] = w1h
    WE[256, C_OUT + 2] = c1h

    XT1h = XT1.astype(np.float16)
    WEh = WE.astype(np.float16)
    A16 = (np.asarray(adj_matrix, dtype=np.float16)
           * np.float16(CP))  # {0, fl16(CP)} exactly

    in_maps = []
    for c in range(NCORES):
        in_maps.append(
            {
                "xt1": XT1h,
                "xt1l": np.ascontiguousarray(XT1h[:, c * ROWS : (c + 1) * ROWS]),
                "wext": WEh,
                "adjt": np.ascontiguousarray(
                    A16[c * ROWS : (c + 1) * ROWS, :].T
                ),
            }
        )
    return in_maps


def _run(in_maps, trace=False, b_zero=True, debug_out=False):
    key = f"nc_b{int(b_zero)}_d{int(debug_out)}"
    if key not in _CACHE:
        _CACHE[key] = _build_nc(b_zero=b_zero, debug_out=debug_out)
    nc = _CACHE[key]
    res = run_bass_kernel_spmd(
        nc, in_maps, core_ids=list(range(NCORES)), trace=trace
    )
    full = np.concatenate(
        [res.results[c]["out"] for c in range(NCORES)], axis=0
    ).astype(np.float32)
    return full, res


def kernel(node_feats, adj_matrix, W, b, v0, v1):
    in_maps = _prep_inputs(node_feats, adj_matrix, W, b, v0, v1)
    trace = bool(int(os.environ.get("GAT_TRACE", "0")))
    b_zero = not bool(np.any(np.asarray(b)))
    full, _ = _run(in_maps, trace=trace, b_zero=b_zero)
    return full


# revision 17
# speedup vs baseline: 1.1510x; 1.1510x over previous
"""GAT single-head forward on 8 Trainium2 NeuronCores (Bass/Tile).

Math (per reference):
    h   = X @ W + b                      [N, 128]
    f1  = h @ v0, f2 = h @ v1            [N]
    logits = adj * (f1[:,None] + f2[None,:])   (adj entries are exactly 0/1)
    vals = sigmoid(logits) - 0.5
    masked softmax over row edges; out = probs @ h

Identities used on device:
  * On edges the softmax weight is w = exp(sigmoid(s)), s = f1_i + f2_j,
    up to a per-row factor that cancels; with t = tanh(s/2):
    w = exp(0.5*t + 0.5). Tanh and Exp share one activation table set.
  * EXP route (exact): w = exp(0.5 t + 0.5), masked by et = A*w (one fused
    tensor_tensor multiply per group -- 2x DVE mode).
  * QUAD route (NQ j-chunks, offloads the ACT-bound exp to the DVE):
    w ~ rho*[(t+d)^2 + C] (minimax rel err 5.4e-3). Realized as
    u = t + 2d (ts 4x), P = u*t (tt 2x), Pc = P + CPQ (ts 4x),
    et = A*Pc (tt 2x), where CPQ = C + d^2. rho is fixed so the two
    routes agree: rho = 1 (weights from either route are used in the SAME
    softmax, so the quad is fit with rho free and then DIVIDED by rho --
    i.e. constants are pre-scaled so both routes approximate exp(...)
    directly).
  * A ones-column appended to h turns the softmax denominator into one
    extra matmul output column.

Sharding: rows of adj across the 8 cores (1024 rows each). node_feats is
replicated; every core computes the full projected h - no collectives.

Per-core layout: adj block TRANSPOSED ([j=source node on partitions,
i=own rows on free dim]) so the aggregate contracts over the partition
dim. adj is cast to fp16 host-side (exact for a 0/1 mask).

Engine budget per core (8.4M dense elements):
  ACT: tanh (all chunks, fused) + exp (chunks not on the quad route)
  DVE: per-q preadd (ts) + per-group fused mask multiplies (tt) + quad ops
  PE : single aggregate et @ h_ext (512 LDW+MM pairs) + h-projection
"""

import os

import numpy as np

import concourse.mybir as mybir
import concourse.tile as tile
from concourse import bacc
from concourse.bass_utils import run_bass_kernel_spmd

F32 = mybir.dt.float32
F16 = mybir.dt.float16
AF = mybir.ActivationFunctionType
ALU = mybir.AluOpType

N, C_IN, C_OUT = 8192, 256, 128
NCORES = 8
ROWS = N // NCORES          # 1024 rows of adj per core
P = 128
NT = N // P                 # 64 node tiles (j-chunks)
NI = ROWS // P              # 8 output row-tiles per core
KC = [128, 128, 1]          # contraction chunks of K=257
WCOLS = C_OUT + 3           # [W | ones-hack | 0.5*w0 | 0.5*w1]
HCOLS = C_OUT + 1           # h plus the ones column
TINY = float(np.finfo(np.float32).tiny)
BANK = 512                  # PSUM bank, fp32 elements

# quadratic approx of exp((1+t)/2) ~ rho*[(t+D)^2 + CC]; both routes feed
# the same softmax, so divide the quad by rho: P/rho family via pre-scaled
# constants. (t+D)^2 + CC = t^2 + 2D t + (D^2+CC); we emit
# u = t/RQ + 2D/RQ... simpler: u = t + 2D, P = u*t, Pc = P*1 + CPQ, then
# et = A*(Pc) and finally weights differ from exp-route by factor rho ->
# fold 1/rho into Pc via PS (see below).
D = 2.079251
CC = 3.749643
RHO = 0.204586                      # fit scale: exp ~= RHO*((t+D)^2+CC)
# y = RHO*((t+2D)*t) ; et = A*(y + RHO*(D*D+CC))
CPQ = float(RHO * (D * D + CC))
# NQ j-chunks take the quad route (engine balance knob; rest take exp)
NQ = 16

GROUPS = [2, 4] + [6] * 9 + [4]     # j-chunk fusion per ACT/DVE instr

_CACHE: dict = {}


def _quad_chunks():
    """First NS_Q(g) chunks of each group run the quad route (spread so
    both routes stay pipelined)."""
    qs = set()
    q0 = 0
    left = NQ
    for gsz in GROUPS:
        take = min(gsz, max(0, min(left, int(round(gsz * NQ / NT + 0.499)))))
        for k in range(take):
            qs.add(q0 + k)
        left -= take
        q0 += gsz
    # adjust to exactly NQ
    q = 0
    while left > 0 and q < NT:
        if q not in qs:
            qs.add(q); left -= 1
        q += 1
    while left < 0:
        q = max(qs); qs.remove(q); left += 1
    return qs


QSET = _quad_chunks()


def _build_nc(b_zero=True):
    nc = bacc.Bacc(
        "TRN2", target_bir_lowering=False, debug=False, num_devices=NCORES
    )
    xt1 = nc.dram_tensor("xt1", [257, N], F16, kind="ExternalInput").ap()
    xt1l = nc.dram_tensor("xt1l", [257, ROWS], F16, kind="ExternalInput").ap()
    wext = nc.dram_tensor("wext", [257, WCOLS], F16, kind="ExternalInput").ap()
    adjt = nc.dram_tensor("adjt", [N, ROWS], F16, kind="ExternalInput").ap()
    out = nc.dram_tensor("out", [ROWS, C_OUT], F32, kind="ExternalOutput").ap()

    with tile.TileContext(nc) as tc:
        _emit(tc, nc, xt1, xt1l, wext, adjt, out, b_zero)
    nc.compile()
    return nc


def _emit(tc, nc, xt1, xt1l, wext, adjt, out, b_zero):
    from contextlib import ExitStack

    nkc = 2 if b_zero else 3

    with ExitStack() as ctx:
        # ---- persistent tiles ----
        persist = ctx.enter_context(tc.tile_pool(name="persist", bufs=1))
        h16_all = persist.tile([P, NT * HCOLS], F16, tag="h16")   # [128, 8256]
        f2h_all = persist.tile([P, NT], F32, tag="f2h")           # 0.5*f2 per j
        f1rep = persist.tile([P, ROWS], F16, tag="f1rep")         # 0.5*f1 bcast
        zero1 = persist.tile([P, 1], F32, tag="zero1")
        nc.vector.memset(zero1[:], 0.0)
        half1 = persist.tile([P, 1], F32, tag="half1")
        nc.vector.memset(half1[:], 0.5)
        if b_zero:
            nc.vector.memset(
                h16_all[:].rearrange("p (t c) -> p t c", c=HCOLS)[
                    :, :, C_OUT : C_OUT + 1
                ],
                1.0,
            )

        xtp = ctx.enter_context(tc.tile_pool(name="xt", bufs=1))

        # ---- input loads ----
        offs = [0, 128, 256]
        xts = [
            xtp.tile([KC[k], N], F16, name=f"xtsb{k}", tag=f"xt{k}")
            for k in range(nkc)
        ]
        SUBS = [0, 1024, 3072, 5120, N]
        wes, xls = [], []
        off = 0
        for k in range(nkc):
            kc = KC[k]
            wx_sb = xtp.tile([kc, WCOLS + ROWS], F16, name=f"wx{k}", tag=f"wx{k}")
            nc.sync.dma_start(wx_sb[:, 0:WCOLS], wext[off : off + kc, :])
            nc.sync.dma_start(wx_sb[:, WCOLS:], xt1l[off : off + kc, :])
            wes.append(wx_sb[:, 0:WCOLS])
            xls.append(wx_sb[:, WCOLS:])
            off += kc
        for k in range(nkc):
            if KC[k] == P:
                nc.sync.dma_start(
                    xts[k][:, 0 : SUBS[1]],
                    xt1[offs[k] : offs[k] + KC[k], 0 : SUBS[1]],
                )
        for c in range(1, len(SUBS) - 1):
            for k in range(nkc):
                if KC[k] != P:
                    if c == 1:
                        nc.sync.dma_start(
                            xts[k][:], xt1[offs[k] : offs[k] + KC[k], :]
                        )
                    continue
                nc.sync.dma_start(
                    xts[k][:, SUBS[c] : SUBS[c + 1]],
                    xt1[offs[k] : offs[k] + KC[k], SUBS[c] : SUBS[c + 1]],
                )

        # ---- f1 path ----
        with tc.tile_pool(name="pf", bufs=1, space="PSUM") as pfp:
            prep = pfp.tile([P, ROWS], F32, tag="prep")
            for k in range(nkc):
                for nh in range(ROWS // 512):
                    nc.tensor.matmul(
                        prep[:, nh * 512 : (nh + 1) * 512],
                        wes[k][:, C_OUT + 1 : C_OUT + 2].to_broadcast(
                            (KC[k], P)
                        ),
                        xls[k][:, nh * 512 : (nh + 1) * 512],
                        start=(k == 0),
                        stop=(k == nkc - 1),
                    )
            nc.scalar.copy(f1rep[:], prep[:])

        # ---- f2 head start ----
        F2HEAD = 8
        with tc.tile_pool(name="pf2", bufs=1, space="PSUM") as pf2p:
            pt = pf2p.tile([P, NI * BANK], F32, tag="pt")
            pt3 = pt[:].rearrange("p (t w) -> p t w", w=BANK)
            for q in range(F2HEAD):
                w = (q % NI) * BANK
                for k in range(nkc):
                    nc.tensor.matmul(
                        pt[:, w : w + 1],
                        xts[k][:, q * P : (q + 1) * P],
                        wes[k][:, C_OUT + 2 : C_OUT + 3],
                        start=(k == 0),
                        stop=(k == nkc - 1),
                    )
                if q == 1:
                    nc.vector.tensor_copy(
                        f2h_all[:, 0:2], pt3[:, 0:2, 0:1]
                    )
            nc.vector.tensor_copy(
                f2h_all[:, 2:F2HEAD], pt3[:, 2:F2HEAD, 0:1]
            )

        # ---- main-loop pools ----
        atp = ctx.enter_context(tc.tile_pool(name="atp", bufs=3))   # adj
        xtp2 = ctx.enter_context(tc.tile_pool(name="xtp2", bufs=2))  # s/2 -> t
        wtp = ctx.enter_context(tc.tile_pool(name="wtp", bufs=2))   # w / quad
        etp = ctx.enter_context(tc.tile_pool(name="etp", bufs=2))   # masked
        obp = ctx.enter_context(tc.tile_pool(name="ob", bufs=2))

        group_q0 = []
        q0 = 0
        for gsz in GROUPS:
            group_q0.append(q0)
            q0 += gsz

        deferred = []

        def emit_group_front(g):
            """adj DMA, per-q preadd, fused tanh, exp/quad, fused mask."""
            gsz = GROUPS[g]
            q0 = group_q0[g]
            at_sup = atp.tile([P, gsz * ROWS], F16, tag="at", name=f"at{g}")
            nc.sync.dma_start(
                at_sup[:].rearrange("p (q i) -> p q i", i=ROWS),
                adjt.rearrange("(q p) i -> p q i", p=P)[:, q0 : q0 + gsz, :],
            )
            xg = xtp2.tile([P, gsz * ROWS], F16, tag="xg", name=f"xg{g}")
            for qq in range(gsz):
                q = q0 + qq
                nc.vector.tensor_scalar_add(
                    xg[:, qq * ROWS : (qq + 1) * ROWS],
                    f1rep[:],
                    f2h_all[:, q : q + 1],
                )
            # t = tanh(s/2) in place (proven reader-rewriter pattern)
            nc.scalar.activation(xg[:], xg[:], AF.Tanh, bias=zero1[:])
            wg = wtp.tile([P, gsz * ROWS], F16, tag="wg", name=f"wg{g}")
            # routes are split at chunk granularity: quad chunks must be a
            # contiguous prefix/suffix pattern per group for fused ops; we
            # assign per-chunk here, fusing contiguous runs.
            runs = []  # (start_qq, end_qq, is_quad)
            for qq in range(gsz):
                isq = (q0 + qq) in QSET
                if runs and runs[-1][2] == isq:
                    runs[-1][1] = qq + 1
                else:
                    runs.append([qq, qq + 1, isq])
            for r0, r1, isq in runs:
                sl = slice(r0 * ROWS, r1 * ROWS)
                if not isq:
                    # exact: w = exp(0.5 t + 0.5)
                    nc.scalar.activation(
                        wg[:, sl], xg[:, sl], AF.Exp, bias=half1[:], scale=0.5
                    )
            et = etp.tile([P, gsz * ROWS], F16, tag="et", name=f"et{g}")
            for r0, r1, isq in runs:
                sl = slice(r0 * ROWS, r1 * ROWS)
                if isq:
                    # quad: u = RHO*t + 2*D*RHO (ts 4x), y = u*t (tt 2x),
                    # et = (y + CPQ)*A (stt, single writer per range)
                    nc.vector.tensor_scalar(
                        wg[:, sl], xg[:, sl],
                        float(RHO), float(2 * D * RHO), ALU.mult, ALU.add,
                    )
                    yq = wtp.tile(
                        [P, (r1 - r0) * ROWS], F16, tag="yq", name=f"yq{g}"
                    )
                    nc.vector.tensor_mul(yq[:], wg[:, sl], xg[:, sl])
                    nc.vector.scalar_tensor_tensor(
                        et[:, sl], yq[:], CPQ, at_sup[:, sl],
                        ALU.add, ALU.mult,
                    )
                else:
                    # exact route mask: et = A * w
                    nc.vector.tensor_mul(
                        et[:, sl], at_sup[:, sl], wg[:, sl]
                    )
            return {"g": g, "gsz": gsz, "q0": q0, "et": et}

        def emit_group_back(fr, po_all, mid=None):
            gsz, q0, et = fr["gsz"], fr["q0"], fr["et"]
            for qq in range(gsz):
                if qq == min(2, gsz - 1) and mid is not None:
                    mid()
                q = q0 + qq
                rhs = h16_all[:, q * HCOLS : (q + 1) * HCOLS]
                for it in range(NI):
                    nc.tensor.matmul(
                        po_all[:, it * BANK : it * BANK + HCOLS],
                        et[:, qq * ROWS + it * P : qq * ROWS + (it + 1) * P],
                        rhs,
                        start=(q == 0),
                        stop=(q == NT - 1),
                    )

        # ---- h-projection ----
        next_group = 0
        with tc.tile_pool(name="php", bufs=1, space="PSUM") as php:
            ph_all = php.tile([P, NI * BANK], F32, tag="ph")
            for b in range(NT // 4):
                for half in range(2):
                    nt0 = 4 * b + 2 * half
                    w0 = (nt0 % NI) * BANK
                    w1 = ((nt0 + 1) % NI) * BANK
                    for k in range(nkc):
                        nc.tensor.matmul(
                            ph_all[:, w0 : w0 + WCOLS],
                            xts[k][:, nt0 * P : (nt0 + 1) * P],
                            wes[k][:],
                            start=(k == 0),
                            stop=(k == nkc - 1),
                        )
                        nc.tensor.matmul(
                            ph_all[:, w1 : w1 + WCOLS],
                            xts[k][:, (nt0 + 1) * P : (nt0 + 2) * P],
                            wes[k][:],
                            start=(k == 0),
                            stop=(k == nkc - 1),
                        )
                bt = 4 * b
                wlo = (bt % NI) * BANK
                src = ph_all[:, wlo : wlo + 4 * BANK].rearrange(
                    "p (b w) -> p b w", b=4
                )
                dst_h = h16_all[:, bt * HCOLS : (bt + 4) * HCOLS].rearrange(
                    "p (b w) -> p b w", b=4
                )
                hc = C_OUT if b_zero else HCOLS
                nc.vector.tensor_copy(dst_h[:, :, 0:hc], src[:, :, 0:hc])
                if bt >= 8:
                    nc.vector.tensor_copy(
                        f2h_all[:, bt : bt + 4],
                        src[:, :, C_OUT + 2 : C_OUT + 3],
                    )
                while (
                    next_group < len(GROUPS)
                    and group_q0[next_group] + GROUPS[next_group] <= 4 * (b + 1)
                    and len(deferred) < 2
                ):
                    deferred.append(emit_group_front(next_group))
                    next_group += 1

        # ---- aggregate accumulators ----
        pop = ctx.enter_context(tc.tile_pool(name="po", bufs=1, space="PSUM"))
        po_all = pop.tile([P, NI * BANK], F32, tag="poall")

        for g in range(next_group, len(GROUPS)):
            emit_group_back(
                deferred.pop(0),
                po_all,
                mid=lambda g=g: deferred.append(emit_group_front(g)),
            )
        for fr in deferred:
            emit_group_back(fr, po_all)

        # ---- epilogue ----
        ns = obp.tile([P, NI * HCOLS], F32, tag="ns")
        ns3 = ns[:].rearrange("p (t c) -> p t c", c=HCOLS)
        dm = obp.tile([P, NI], F32, tag="dm")
        for it in range(NI):
            if it % 2 == 0:
                nc.vector.tensor_copy(
                    ns3[:, it, :], po_all[:, it * BANK : it * BANK + HCOLS]
                )
            else:
                nc.scalar.copy(
                    ns3[:, it, :], po_all[:, it * BANK : it * BANK + HCOLS]
                )
            nc.vector.tensor_scalar_max(
                dm[:, it : it + 1], ns3[:, it, C_OUT : C_OUT + 1], TINY
            )
        rc = obp.tile([P, NI], F32, tag="rc")
        nc.vector.reciprocal(rc[:], dm[:])
        ob_all = obp.tile([P, NI * C_OUT], F32, tag="oball")
        for it in range(NI):
            if it % 2 == 0:
                nc.vector.tensor_scalar_mul(
                    ob_all[:, it * C_OUT : (it + 1) * C_OUT],
                    ns3[:, it, 0:C_OUT],
                    rc[:, it : it + 1],
                )
            else:
                nc.scalar.mul(
                    ob_all[:, it * C_OUT : (it + 1) * C_OUT],
                    ns3[:, it, 0:C_OUT],
                    rc[:, it : it + 1],
                )
        nc.sync.dma_start(
            out.rearrange("(t p) c -> p t c", p=P),
            ob_all[:].rearrange("p (t c) -> p t c", c=C_OUT),
        )


def _prep_inputs(node_feats, adj_matrix, W, b, v0, v1):
    X = np.ascontiguousarray(node_feats, dtype=np.float32)
    W = np.asarray(W, dtype=np.float32)
    b = np.asarray(b, dtype=np.float32)
    v0 = np.asarray(v0, dtype=np.float32)
    v1 = np.asarray(v1, dtype=np.float32)

    w0h = (0.5 * (W.astype(np.float64) @ v0.astype(np.float64))).astype(np.float32)
    w1h = (0.5 * (W.astype(np.float64) @ v1.astype(np.float64))).astype(np.float32)
    c0h = np.float32(0.5 * float(b.astype(np.float64) @ v0.astype(np.float64)))
    c1h = np.float32(0.5 * float(b.astype(np.float64) @ v1.astype(np.float64)))

    XT1 = np.empty((257, N), np.float32)
    XT1[:256] = X.T
    XT1[256] = 1.0

    WE = np.zeros((257, WCOLS), np.float32)
    WE[:256, :C_OUT] = W
    WE[256, :C_OUT] = b
    WE[256, C_OUT] = 1.0
    WE[:256, C_OUT + 1] = w0h
    WE[256, C_OUT + 1] = c0h
    WE[:256, C_OUT + 2] = w1h
    WE[256, C_OUT + 2] = c1h

    XT1h = XT1.astype(np.float16)
    WEh = WE.astype(np.float16)
    A16 = np.asarray(adj_matrix, dtype=np.float16)

    in_maps = []
    for c in range(NCORES):
        in_maps.append(
            {
                "xt1": XT1h,
                "xt1l": np.ascontiguousarray(XT1h[:, c * ROWS : (c + 1) * ROWS]),
                "wext": WEh,
                "adjt": np.ascontiguousarray(
                    A16[c * ROWS : (c + 1) * ROWS, :].T
                ),
            }
        )
    return in_maps


def _run(in_maps, trace=False, b_zero=True):
    key = f"nc_b{int(b_zero)}"
    if key not in _CACHE:
        _CACHE[key] = _build_nc(b_zero=b_zero)
    nc = _CACHE[key]
    res = run_bass_kernel_spmd(
        nc, in_maps, core_ids=list(range(NCORES)), trace=trace
    )
    full = np.concatenate(
        [res.results[c]["out"] for c in range(NCORES)], axis=0
    ).astype(np.float32)
    return full, res


def kernel(node_feats, adj_matrix, W, b, v0, v1):
    in_maps = _prep_inputs(node_feats, adj_matrix, W, b, v0, v1)
    trace = bool(int(os.environ.get("GAT_TRACE", "0")))
    b_zero = not bool(np.any(np.asarray(b)))
    full, _ = _run(in_maps, trace=trace, b_zero=b_zero)
    return full


# revision 19
# speedup vs baseline: 1.2530x; 1.0886x over previous
"""GAT single-head forward on 8 Trainium2 NeuronCores (Bass/Tile).

Math (per reference):
    h   = X @ W + b                      [N, 128]
    f1  = h @ v0, f2 = h @ v1            [N]
    logits = adj * (f1[:,None] + f2[None,:])   (adj entries are exactly 0/1)
    vals = sigmoid(logits) - 0.5
    masked softmax over row edges; out = probs @ h

Identities used on device:
  * On edges the softmax weight is w = exp(sigmoid(s)), s = f1_i + f2_j,
    up to a per-row factor that cancels; with t = tanh(s/2):
    w = exp(0.5*t + 0.5). Tanh and Exp share one activation table set.
  * EXP route (exact): w = exp(0.5 t + 0.5), masked by et = A*w (one fused
    tensor_tensor multiply per group -- 2x DVE mode).
  * QUAD route (NQ j-chunks, offloads the ACT-bound exp to the DVE):
    w ~ rho*[(t+d)^2 + C] (minimax rel err 5.4e-3). Realized as
    u = t + 2d (ts 4x), P = u*t (tt 2x), Pc = P + CPQ (ts 4x),
    et = A*Pc (tt 2x), where CPQ = C + d^2. rho is fixed so the two
    routes agree: rho = 1 (weights from either route are used in the SAME
    softmax, so the quad is fit with rho free and then DIVIDED by rho --
    i.e. constants are pre-scaled so both routes approximate exp(...)
    directly).
  * A ones-column appended to h turns the softmax denominator into one
    extra matmul output column.

Sharding: rows of adj across the 8 cores (1024 rows each). node_feats is
replicated; every core computes the full projected h - no collectives.

Per-core layout: adj block TRANSPOSED ([j=source node on partitions,
i=own rows on free dim]) so the aggregate contracts over the partition
dim. adj is cast to fp16 host-side (exact for a 0/1 mask).

Engine budget per core (8.4M dense elements):
  ACT: tanh (all chunks, fused) + exp (chunks not on the quad route)
  DVE: per-q preadd (ts) + per-group fused mask multiplies (tt) + quad ops
  PE : single aggregate et @ h_ext (512 LDW+MM pairs) + h-projection
"""

import os

import numpy as np

import concourse.mybir as mybir
import concourse.tile as tile
from concourse import bacc
from concourse.bass_utils import run_bass_kernel_spmd

F32 = mybir.dt.float32
F16 = mybir.dt.float16
AF = mybir.ActivationFunctionType
ALU = mybir.AluOpType

N, C_IN, C_OUT = 8192, 256, 128
NCORES = 8
ROWS = N // NCORES          # 1024 rows of adj per core
P = 128
NT = N // P                 # 64 node tiles (j-chunks)
NI = ROWS // P              # 8 output row-tiles per core
KC = [128, 128, 1]          # contraction chunks of K=257
WCOLS = C_OUT + 3           # [W | ones-hack | 0.5*w0 | 0.5*w1]
HCOLS = C_OUT + 1           # h plus the ones column
TINY = float(np.finfo(np.float32).tiny)
BANK = 512                  # PSUM bank, fp32 elements

# quadratic approx of exp((1+t)/2) ~ rho*[(t+D)^2 + CC]; both routes feed
# the same softmax, so divide the quad by rho: P/rho family via pre-scaled
# constants. (t+D)^2 + CC = t^2 + 2D t + (D^2+CC); we emit
# u = t/RQ + 2D/RQ... simpler: u = t + 2D, P = u*t, Pc = P*1 + CPQ, then
# et = A*(Pc) and finally weights differ from exp-route by factor rho ->
# fold 1/rho into Pc via PS (see below).
D = 2.079251
CC = 3.749643
RHO = 0.204586                      # fit scale: exp ~= RHO*((t+D)^2+CC)
# y = RHO*((t+2D)*t) ; et = A*(y + RHO*(D*D+CC))
CPQ = float(RHO * (D * D + CC))
# NQ j-chunks take the quad route (engine balance knob; rest take exp)
NQ = 16

GROUPS = [2, 4] + [6] * 9 + [4]     # j-chunk fusion per ACT/DVE instr

_CACHE: dict = {}


def _quad_chunks():
    """First NS_Q(g) chunks of each group run the quad route (spread so
    both routes stay pipelined)."""
    qs = set()
    q0 = 0
    left = NQ
    for gsz in GROUPS:
        take = min(gsz, max(0, min(left, int(round(gsz * NQ / NT + 0.499)))))
        for k in range(take):
            qs.add(q0 + k)
        left -= take
        q0 += gsz
    # adjust to exactly NQ
    q = 0
    while left > 0 and q < NT:
        if q not in qs:
            qs.add(q); left -= 1
        q += 1
    while left < 0:
        q = max(qs); qs.remove(q); left += 1
    return qs


QSET = _quad_chunks()


def _build_nc(b_zero=True):
    nc = bacc.Bacc(
        "TRN2", target_bir_lowering=False, debug=False, num_devices=NCORES
    )
    xt1 = nc.dram_tensor("xt1", [257, N], F16, kind="ExternalInput").ap()
    xt1l = nc.dram_tensor("xt1l", [257, ROWS], F16, kind="ExternalInput").ap()
    wext = nc.dram_tensor("wext", [257, WCOLS], F16, kind="ExternalInput").ap()
    adjt = nc.dram_tensor("adjt", [N, ROWS], F16, kind="ExternalInput").ap()
    out = nc.dram_tensor("out", [ROWS, C_OUT], F32, kind="ExternalOutput").ap()

    with tile.TileContext(nc) as tc:
        _emit(tc, nc, xt1, xt1l, wext, adjt, out, b_zero)
    nc.compile()
    return nc


def _emit(tc, nc, xt1, xt1l, wext, adjt, out, b_zero):
    from contextlib import ExitStack

    nkc = 2 if b_zero else 3

    with ExitStack() as ctx:
        # ---- persistent tiles ----
        persist = ctx.enter_context(tc.tile_pool(name="persist", bufs=1))
        h16_all = persist.tile([P, NT * HCOLS], F16, tag="h16")   # [128, 8256]
        f2h_all = persist.tile([P, NT], F32, tag="f2h")           # 0.5*f2 per j
        f1rep = persist.tile([P, ROWS], F16, tag="f1rep")         # 0.5*f1 bcast
        zero1 = persist.tile([P, 1], F32, tag="zero1")
        nc.vector.memset(zero1[:], 0.0)
        half1 = persist.tile([P, 1], F32, tag="half1")
        nc.vector.memset(half1[:], 0.5)
        if b_zero:
            nc.vector.memset(
                h16_all[:].rearrange("p (t c) -> p t c", c=HCOLS)[
                    :, :, C_OUT : C_OUT + 1
                ],
                1.0,
            )

        xtp = ctx.enter_context(tc.tile_pool(name="xt", bufs=1))

        # ---- input loads ----
        offs = [0, 128, 256]
        xts = [
            xtp.tile([KC[k], N], F16, name=f"xtsb{k}", tag=f"xt{k}")
            for k in range(nkc)
        ]
        SUBS = [0, 1024, 3072, 5120, N]
        wes, xls = [], []
        off = 0
        for k in range(nkc):
            kc = KC[k]
            wx_sb = xtp.tile([kc, WCOLS + ROWS], F16, name=f"wx{k}", tag=f"wx{k}")
            nc.sync.dma_start(wx_sb[:, 0:WCOLS], wext[off : off + kc, :])
            nc.sync.dma_start(wx_sb[:, WCOLS:], xt1l[off : off + kc, :])
            wes.append(wx_sb[:, 0:WCOLS])
            xls.append(wx_sb[:, WCOLS:])
            off += kc
        for k in range(nkc):
            if KC[k] == P:
                nc.sync.dma_start(
                    xts[k][:, 0 : SUBS[1]],
                    xt1[offs[k] : offs[k] + KC[k], 0 : SUBS[1]],
                )
        for c in range(1, len(SUBS) - 1):
            for k in range(nkc):
                if KC[k] != P:
                    if c == 1:
                        nc.sync.dma_start(
                            xts[k][:], xt1[offs[k] : offs[k] + KC[k], :]
                        )
                    continue
                nc.sync.dma_start(
                    xts[k][:, SUBS[c] : SUBS[c + 1]],
                    xt1[offs[k] : offs[k] + KC[k], SUBS[c] : SUBS[c + 1]],
                )

        # ---- f1 path ----
        with tc.tile_pool(name="pf", bufs=1, space="PSUM") as pfp:
            prep = pfp.tile([P, ROWS], F32, tag="prep")
            for k in range(nkc):
                for nh in range(ROWS // 512):
                    nc.tensor.matmul(
                        prep[:, nh * 512 : (nh + 1) * 512],
                        wes[k][:, C_OUT + 1 : C_OUT + 2].to_broadcast(
                            (KC[k], P)
                        ),
                        xls[k][:, nh * 512 : (nh + 1) * 512],
                        start=(k == 0),
                        stop=(k == nkc - 1),
                    )
            nc.scalar.copy(f1rep[:], prep[:])

        # ---- f2 head start ----
        F2HEAD = 8
        with tc.tile_pool(name="pf2", bufs=1, space="PSUM") as pf2p:
            pt = pf2p.tile([P, NI * BANK], F32, tag="pt")
            pt3 = pt[:].rearrange("p (t w) -> p t w", w=BANK)
            for q in range(F2HEAD):
                w = (q % NI) * BANK
                for k in range(nkc):
                    nc.tensor.matmul(
                        pt[:, w : w + 1],
                        xts[k][:, q * P : (q + 1) * P],
                        wes[k][:, C_OUT + 2 : C_OUT + 3],
                        start=(k == 0),
                        stop=(k == nkc - 1),
                    )
                if q == 1:
                    nc.vector.tensor_copy(
                        f2h_all[:, 0:2], pt3[:, 0:2, 0:1]
                    )
            nc.vector.tensor_copy(
                f2h_all[:, 2:F2HEAD], pt3[:, 2:F2HEAD, 0:1]
            )

        # ---- main-loop pools ----
        atp = ctx.enter_context(tc.tile_pool(name="atp", bufs=3))   # adj
        xtp2 = ctx.enter_context(tc.tile_pool(name="xtp2", bufs=2))  # s/2 -> t
        wtp = ctx.enter_context(tc.tile_pool(name="wtp", bufs=2))   # w / quad
        etp = ctx.enter_context(tc.tile_pool(name="etp", bufs=2))   # masked
        obp = ctx.enter_context(tc.tile_pool(name="ob", bufs=2))

        group_q0 = []
        q0 = 0
        for gsz in GROUPS:
            group_q0.append(q0)
            q0 += gsz

        fa_list = []   # stage-A done (tiles through exp)
        fb_list = []   # stage-B done (masked et ready)

        def emit_group_a(g):
            """adj DMA, per-q preadds (DVE), fused tanh + exp runs (ACT)."""
            gsz = GROUPS[g]
            q0 = group_q0[g]
            at_sup = atp.tile([P, gsz * ROWS], F16, tag="at", name=f"at{g}")
            nc.sync.dma_start(
                at_sup[:].rearrange("p (q i) -> p q i", i=ROWS),
                adjt.rearrange("(q p) i -> p q i", p=P)[:, q0 : q0 + gsz, :],
            )
            xg = xtp2.tile([P, gsz * ROWS], F16, tag="xg", name=f"xg{g}")
            for qq in range(gsz):
                q = q0 + qq
                nc.vector.tensor_scalar_add(
                    xg[:, qq * ROWS : (qq + 1) * ROWS],
                    f1rep[:],
                    f2h_all[:, q : q + 1],
                )
            # t = tanh(s/2) in place (proven reader-rewriter pattern)
            nc.scalar.activation(xg[:], xg[:], AF.Tanh, bias=zero1[:])
            wg = wtp.tile([P, gsz * ROWS], F16, tag="wg", name=f"wg{g}")
            runs = []  # (start_qq, end_qq, is_quad)
            for qq in range(gsz):
                isq = (q0 + qq) in QSET
                if runs and runs[-1][2] == isq:
                    runs[-1][1] = qq + 1
                else:
                    runs.append([qq, qq + 1, isq])
            for r0, r1, isq in runs:
                sl = slice(r0 * ROWS, r1 * ROWS)
                if not isq:
                    # exact route: w = exp(0.5 t + 0.5)
                    nc.scalar.activation(
                        wg[:, sl], xg[:, sl], AF.Exp, bias=half1[:], scale=0.5
                    )
            return {"g": g, "gsz": gsz, "q0": q0, "at": at_sup,
                    "xg": xg, "wg": wg, "runs": runs}

        def emit_group_b(fr):
            """quad-route DVE ops + mask multiplies."""
            gsz, q0 = fr["gsz"], fr["q0"]
            at_sup, xg, wg, runs = fr["at"], fr["xg"], fr["wg"], fr["runs"]
            et = etp.tile([P, gsz * ROWS], F16, tag="et", name=f"et{fr['g']}")
            for r0, r1, isq in runs:
                sl = slice(r0 * ROWS, r1 * ROWS)
                if isq:
                    # quad: u = RHO*t + 2*D*RHO (ts 4x), y = u*t (tt 2x),
                    # et = (y + CPQ)*A (stt, single writer per range)
                    nc.vector.tensor_scalar(
                        wg[:, sl], xg[:, sl],
                        float(RHO), float(2 * D * RHO), ALU.mult, ALU.add,
                    )
                    yq = wtp.tile(
                        [P, (r1 - r0) * ROWS], F16, tag="yq", name=f"yq{fr['g']}"
                    )
                    nc.vector.tensor_mul(yq[:], wg[:, sl], xg[:, sl])
                    nc.vector.scalar_tensor_tensor(
                        et[:, sl], yq[:], CPQ, at_sup[:, sl],
                        ALU.add, ALU.mult,
                    )
                else:
                    # exact route mask: et = A * w
                    nc.vector.tensor_mul(
                        et[:, sl], at_sup[:, sl], wg[:, sl]
                    )
            return {"g": fr["g"], "gsz": gsz, "q0": q0, "et": et}

        def emit_group_back(fr, po_all, mid=None):
            gsz, q0, et = fr["gsz"], fr["q0"], fr["et"]
            for qq in range(gsz):
                if qq == min(2, gsz - 1) and mid is not None:
                    mid()
                q = q0 + qq
                rhs = h16_all[:, q * HCOLS : (q + 1) * HCOLS]
                for it in range(NI):
                    nc.tensor.matmul(
                        po_all[:, it * BANK : it * BANK + HCOLS],
                        et[:, qq * ROWS + it * P : qq * ROWS + (it + 1) * P],
                        rhs,
                        start=(q == 0),
                        stop=(q == NT - 1),
                    )

        # ---- h-projection ----
        next_group = 0
        with tc.tile_pool(name="php", bufs=1, space="PSUM") as php:
            ph_all = php.tile([P, NI * BANK], F32, tag="ph")
            for b in range(NT // 4):
                for half in range(2):
                    nt0 = 4 * b + 2 * half
                    w0 = (nt0 % NI) * BANK
                    w1 = ((nt0 + 1) % NI) * BANK
                    for k in range(nkc):
                        nc.tensor.matmul(
                            ph_all[:, w0 : w0 + WCOLS],
                            xts[k][:, nt0 * P : (nt0 + 1) * P],
                            wes[k][:],
                            start=(k == 0),
                            stop=(k == nkc - 1),
                        )
                        nc.tensor.matmul(
                            ph_all[:, w1 : w1 + WCOLS],
                            xts[k][:, (nt0 + 1) * P : (nt0 + 2) * P],
                            wes[k][:],
                            start=(k == 0),
                            stop=(k == nkc - 1),
                        )
                bt = 4 * b
                wlo = (bt % NI) * BANK
                src = ph_all[:, wlo : wlo + 4 * BANK].rearrange(
                    "p (b w) -> p b w", b=4
                )
                dst_h = h16_all[:, bt * HCOLS : (bt + 4) * HCOLS].rearrange(
                    "p (b w) -> p b w", b=4
                )
                hc = C_OUT if b_zero else HCOLS
                nc.vector.tensor_copy(dst_h[:, :, 0:hc], src[:, :, 0:hc])
                if bt >= 8:
                    nc.vector.tensor_copy(
                        f2h_all[:, bt : bt + 4],
                        src[:, :, C_OUT + 2 : C_OUT + 3],
                    )
                while (
                    next_group < len(GROUPS)
                    and group_q0[next_group] + GROUPS[next_group] <= 4 * (b + 1)
                    and len(fa_list) + len(fb_list) < 2
                ):
                    fa_list.append(emit_group_a(next_group))
                    next_group += 1
                if len(fa_list) >= 2 and not fb_list:
                    fb_list.append(emit_group_b(fa_list.pop(0)))

        # ---- aggregate accumulators ----
        pop = ctx.enter_context(tc.tile_pool(name="po", bufs=1, space="PSUM"))
        po_all = pop.tile([P, NI * BANK], F32, tag="poall")

        # steady pipeline: back(g) mid-emits stage-A(g+2) then stage-B(g+1)
        def advance_a():
            nonlocal next_group
            if next_group < len(GROUPS):
                fa_list.append(emit_group_a(next_group))
                next_group += 1

        def advance_b():
            if fa_list:
                fb_list.append(emit_group_b(fa_list.pop(0)))

        while not fb_list:
            if not fa_list:
                advance_a()
            advance_b()
        while fb_list:
            fr = fb_list.pop(0)
            emit_group_back(
                fr,
                po_all,
                mid=lambda: (advance_a(), advance_b()),
            )

        # ---- epilogue ----
        ns = obp.tile([P, NI * HCOLS], F32, tag="ns")
        ns3 = ns[:].rearrange("p (t c) -> p t c", c=HCOLS)
        dm = obp.tile([P, NI], F32, tag="dm")
        for it in range(NI):
            if it % 2 == 0:
                nc.vector.tensor_copy(
                    ns3[:, it, :], po_all[:, it * BANK : it * BANK + HCOLS]
                )
            else:
                nc.scalar.copy(
                    ns3[:, it, :], po_all[:, it * BANK : it * BANK + HCOLS]
                )
            nc.vector.tensor_scalar_max(
                dm[:, it : it + 1], ns3[:, it, C_OUT : C_OUT + 1], TINY
            )
        rc = obp.tile([P, NI], F32, tag="rc")
        nc.vector.reciprocal(rc[:], dm[:])
        ob_all = obp.tile([P, NI * C_OUT], F32, tag="oball")
        for it in range(NI):
            if it % 2 == 0:
                nc.vector.tensor_scalar_mul(
                    ob_all[:, it * C_OUT : (it + 1) * C_OUT],
                    ns3[:, it, 0:C_OUT],
                    rc[:, it : it + 1],
                )
            else:
                nc.scalar.mul(
                    ob_all[:, it * C_OUT : (it + 1) * C_OUT],
                    ns3[:, it, 0:C_OUT],
                    rc[:, it : it + 1],
                )
        nc.sync.dma_start(
            out.rearrange("(t p) c -> p t c", p=P),
            ob_all[:].rearrange("p (t c) -> p t c", c=C_OUT),
        )


def _prep_inputs(node_feats, adj_matrix, W, b, v0, v1):
    X = np.ascontiguousarray(node_feats, dtype=np.float32)
    W = np.asarray(W, dtype=np.float32)
    b = np.asarray(b, dtype=np.float32)
    v0 = np.asarray(v0, dtype=np.float32)
    v1 = np.asarray(v1, dtype=np.float32)

    w0h = (0.5 * (W.astype(np.float64) @ v0.astype(np.float64))).astype(np.float32)
    w1h = (0.5 * (W.astype(np.float64) @ v1.astype(np.float64))).astype(np.float32)
    c0h = np.float32(0.5 * float(b.astype(np.float64) @ v0.astype(np.float64)))
    c1h = np.float32(0.5 * float(b.astype(np.float64) @ v1.astype(np.float64)))

    XT1 = np.empty((257, N), np.float32)
    XT1[:256] = X.T
    XT1[256] = 1.0

    WE = np.zeros((257, WCOLS), np.float32)
    WE[:256, :C_OUT] = W
    WE[256, :C_OUT] = b
    WE[256, C_OUT] = 1.0
    WE[:256, C_OUT + 1] = w0h
    WE[256, C_OUT + 1] = c0h
    WE[:256, C_OUT + 2] = w1h
    WE[256, C_OUT + 2] = c1h

    XT1h = XT1.astype(np.float16)
    WEh = WE.astype(np.float16)
    A16 = np.asarray(adj_matrix, dtype=np.float16)

    in_maps = []
    for c in range(NCORES):
        in_maps.append(
            {
                "xt1": XT1h,
                "xt1l": np.ascontiguousarray(XT1h[:, c * ROWS : (c + 1) * ROWS]),
                "wext": WEh,
                "adjt": np.ascontiguousarray(
                    A16[c * ROWS : (c + 1) * ROWS, :].T
                ),
            }
        )
    return in_maps


def _run(in_maps, trace=False, b_zero=True):
    key = f"nc_b{int(b_zero)}"
    if key not in _CACHE:
        _CACHE[key] = _build_nc(b_zero=b_zero)
    nc = _CACHE[key]
    res = run_bass_kernel_spmd(
        nc, in_maps, core_ids=list(range(NCORES)), trace=trace
    )
    full = np.concatenate(
        [res.results[c]["out"] for c in range(NCORES)], axis=0
    ).astype(np.float32)
    return full, res


def kernel(node_feats, adj_matrix, W, b, v0, v1):
    in_maps = _prep_inputs(node_feats, adj_matrix, W, b, v0, v1)
    trace = bool(int(os.environ.get("GAT_TRACE", "0")))
    b_zero = not bool(np.any(np.asarray(b)))
    full, _ = _run(in_maps, trace=trace, b_zero=b_zero)
    return full


# revision 20
# speedup vs baseline: 1.2734x; 1.0163x over previous
"""GAT single-head forward on 8 Trainium2 NeuronCores (Bass/Tile).

Math (per reference):
    h   = X @ W + b                      [N, 128]
    f1  = h @ v0, f2 = h @ v1            [N]
    logits = adj * (f1[:,None] + f2[None,:])   (adj entries are exactly 0/1)
    vals = sigmoid(logits) - 0.5
    masked softmax over row edges; out = probs @ h

Identities used on device:
  * On edges the softmax weight is w = exp(sigmoid(s)), s = f1_i + f2_j,
    up to a per-row factor that cancels; with t = tanh(s/2):
    w = exp(0.5*t + 0.5). Tanh and Exp share one activation table set.
  * EXP route (exact): w = exp(0.5 t + 0.5), masked by et = A*w (one fused
    tensor_tensor multiply per group -- 2x DVE mode).
  * QUAD route (NQ j-chunks, offloads the ACT-bound exp to the DVE):
    w ~ rho*[(t+d)^2 + C] (minimax rel err 5.4e-3). Realized as
    u = t + 2d (ts 4x), P = u*t (tt 2x), Pc = P + CPQ (ts 4x),
    et = A*Pc (tt 2x), where CPQ = C + d^2. rho is fixed so the two
    routes agree: rho = 1 (weights from either route are used in the SAME
    softmax, so the quad is fit with rho free and then DIVIDED by rho --
    i.e. constants are pre-scaled so both routes approximate exp(...)
    directly).
  * A ones-column appended to h turns the softmax denominator into one
    extra matmul output column.

Sharding: rows of adj across the 8 cores (1024 rows each). node_feats is
replicated; every core computes the full projected h - no collectives.

Per-core layout: adj block TRANSPOSED ([j=source node on partitions,
i=own rows on free dim]) so the aggregate contracts over the partition
dim. adj is cast to fp16 host-side (exact for a 0/1 mask).

Engine budget per core (8.4M dense elements):
  ACT: tanh (all chunks, fused) + exp (chunks not on the quad route)
  DVE: per-q preadd (ts) + per-group fused mask multiplies (tt) + quad ops
  PE : single aggregate et @ h_ext (512 LDW+MM pairs) + h-projection
"""

import os

import numpy as np

import concourse.mybir as mybir
import concourse.tile as tile
from concourse import bacc
from concourse.bass_utils import run_bass_kernel_spmd

F32 = mybir.dt.float32
F16 = mybir.dt.float16
AF = mybir.ActivationFunctionType
ALU = mybir.AluOpType

N, C_IN, C_OUT = 8192, 256, 128
NCORES = 8
ROWS = N // NCORES          # 1024 rows of adj per core
P = 128
NT = N // P                 # 64 node tiles (j-chunks)
NI = ROWS // P              # 8 output row-tiles per core
KC = [128, 128, 1]          # contraction chunks of K=257
WCOLS = C_OUT + 3           # [W | ones-hack | 0.5*w0 | 0.5*w1]
HCOLS = C_OUT + 1           # h plus the ones column
TINY = float(np.finfo(np.float32).tiny)
BANK = 512                  # PSUM bank, fp32 elements

# quadratic approx of exp((1+t)/2) ~ rho*[(t+D)^2 + CC]; both routes feed
# the same softmax, so divide the quad by rho: P/rho family via pre-scaled
# constants. (t+D)^2 + CC = t^2 + 2D t + (D^2+CC); we emit
# u = t/RQ + 2D/RQ... simpler: u = t + 2D, P = u*t, Pc = P*1 + CPQ, then
# et = A*(Pc) and finally weights differ from exp-route by factor rho ->
# fold 1/rho into Pc via PS (see below).
D = 2.079251
CC = 3.749643
RHO = 0.204586                      # fit scale: exp ~= RHO*((t+D)^2+CC)
# y = RHO*((t+2D)*t) ; et = A*(y + RHO*(D*D+CC))
CPQ = float(RHO * (D * D + CC))
# NQ j-chunks take the quad route (engine balance knob; rest take exp)
NQ = 18

GROUPS = [4] + [6] * 9 + [2, 2, 2]  # j-chunk fusion per ACT/DVE instr

_CACHE: dict = {}


def _quad_chunks():
    """First NS_Q(g) chunks of each group run the quad route (spread so
    both routes stay pipelined)."""
    qs = set()
    q0 = 0
    left = NQ
    for gsz in GROUPS:
        take = min(gsz, max(0, min(left, int(round(gsz * NQ / NT + 0.499)))))
        for k in range(take):
            qs.add(q0 + k)
        left -= take
        q0 += gsz
    # adjust to exactly NQ
    q = 0
    while left > 0 and q < NT:
        if q not in qs:
            qs.add(q); left -= 1
        q += 1
    while left < 0:
        q = max(qs); qs.remove(q); left += 1
    return qs


QSET = _quad_chunks()


def _build_nc(b_zero=True):
    nc = bacc.Bacc(
        "TRN2", target_bir_lowering=False, debug=False, num_devices=NCORES
    )
    xt1 = nc.dram_tensor("xt1", [257, N], F16, kind="ExternalInput").ap()
    xt1l = nc.dram_tensor("xt1l", [257, ROWS], F16, kind="ExternalInput").ap()
    wext = nc.dram_tensor("wext", [257, WCOLS], F16, kind="ExternalInput").ap()
    adjt = nc.dram_tensor("adjt", [N, ROWS], F16, kind="ExternalInput").ap()
    out = nc.dram_tensor("out", [ROWS, C_OUT], F32, kind="ExternalOutput").ap()

    with tile.TileContext(nc) as tc:
        _emit(tc, nc, xt1, xt1l, wext, adjt, out, b_zero)
    nc.compile()
    return nc


def _emit(tc, nc, xt1, xt1l, wext, adjt, out, b_zero):
    from contextlib import ExitStack

    nkc = 2 if b_zero else 3

    with ExitStack() as ctx:
        # ---- persistent tiles ----
        persist = ctx.enter_context(tc.tile_pool(name="persist", bufs=1))
        h16_all = persist.tile([P, NT * HCOLS], F16, tag="h16")   # [128, 8256]
        f2h_all = persist.tile([P, NT], F32, tag="f2h")           # 0.5*f2 per j
        f1rep = persist.tile([P, ROWS], F16, tag="f1rep")         # 0.5*f1 bcast
        zero1 = persist.tile([P, 1], F32, tag="zero1")
        nc.vector.memset(zero1[:], 0.0)
        half1 = persist.tile([P, 1], F32, tag="half1")
        nc.vector.memset(half1[:], 0.5)
        if b_zero:
            nc.vector.memset(
                h16_all[:].rearrange("p (t c) -> p t c", c=HCOLS)[
                    :, :, C_OUT : C_OUT + 1
                ],
                1.0,
            )

        xtp = ctx.enter_context(tc.tile_pool(name="xt", bufs=1))

        # ---- input loads ----
        offs = [0, 128, 256]
        xts = [
            xtp.tile([KC[k], N], F16, name=f"xtsb{k}", tag=f"xt{k}")
            for k in range(nkc)
        ]
        SUBS = [0, 1024, 3072, 5120, N]
        wes, xls = [], []
        off = 0
        for k in range(nkc):
            kc = KC[k]
            wx_sb = xtp.tile([kc, WCOLS + ROWS], F16, name=f"wx{k}", tag=f"wx{k}")
            nc.sync.dma_start(wx_sb[:, 0:WCOLS], wext[off : off + kc, :])
            nc.sync.dma_start(wx_sb[:, WCOLS:], xt1l[off : off + kc, :])
            wes.append(wx_sb[:, 0:WCOLS])
            xls.append(wx_sb[:, WCOLS:])
            off += kc
        for k in range(nkc):
            if KC[k] == P:
                nc.sync.dma_start(
                    xts[k][:, 0 : SUBS[1]],
                    xt1[offs[k] : offs[k] + KC[k], 0 : SUBS[1]],
                )
        for c in range(1, len(SUBS) - 1):
            for k in range(nkc):
                if KC[k] != P:
                    if c == 1:
                        nc.sync.dma_start(
                            xts[k][:], xt1[offs[k] : offs[k] + KC[k], :]
                        )
                    continue
                nc.sync.dma_start(
                    xts[k][:, SUBS[c] : SUBS[c + 1]],
                    xt1[offs[k] : offs[k] + KC[k], SUBS[c] : SUBS[c + 1]],
                )

        # ---- f1 path ----
        with tc.tile_pool(name="pf", bufs=1, space="PSUM") as pfp:
            prep = pfp.tile([P, ROWS], F32, tag="prep")
            for k in range(nkc):
                for nh in range(ROWS // 512):
                    nc.tensor.matmul(
                        prep[:, nh * 512 : (nh + 1) * 512],
                        wes[k][:, C_OUT + 1 : C_OUT + 2].to_broadcast(
                            (KC[k], P)
                        ),
                        xls[k][:, nh * 512 : (nh + 1) * 512],
                        start=(k == 0),
                        stop=(k == nkc - 1),
                    )
            nc.scalar.copy(f1rep[:], prep[:])

        # ---- f2 head start ----
        F2HEAD = 8
        with tc.tile_pool(name="pf2", bufs=1, space="PSUM") as pf2p:
            pt = pf2p.tile([P, NI * BANK], F32, tag="pt")
            pt3 = pt[:].rearrange("p (t w) -> p t w", w=BANK)
            for q in range(F2HEAD):
                w = (q % NI) * BANK
                for k in range(nkc):
                    nc.tensor.matmul(
                        pt[:, w : w + 1],
                        xts[k][:, q * P : (q + 1) * P],
                        wes[k][:, C_OUT + 2 : C_OUT + 3],
                        start=(k == 0),
                        stop=(k == nkc - 1),
                    )
                if q == 1:
                    nc.vector.tensor_copy(
                        f2h_all[:, 0:2], pt3[:, 0:2, 0:1]
                    )
            nc.vector.tensor_copy(
                f2h_all[:, 2:F2HEAD], pt3[:, 2:F2HEAD, 0:1]
            )

        # ---- main-loop pools ----
        atp = ctx.enter_context(tc.tile_pool(name="atp", bufs=3))   # adj
        xtp2 = ctx.enter_context(tc.tile_pool(name="xtp2", bufs=2))  # s/2 -> t
        wtp = ctx.enter_context(tc.tile_pool(name="wtp", bufs=2))   # w / quad
        etp = ctx.enter_context(tc.tile_pool(name="etp", bufs=2))   # masked
        obp = ctx.enter_context(tc.tile_pool(name="ob", bufs=2))

        group_q0 = []
        q0 = 0
        for gsz in GROUPS:
            group_q0.append(q0)
            q0 += gsz

        fa_list = []   # stage-A done (tiles through exp)
        fb_list = []   # stage-B done (masked et ready)

        def emit_group_a(g):
            """adj DMA, per-q preadds (DVE), fused tanh + exp runs (ACT)."""
            gsz = GROUPS[g]
            q0 = group_q0[g]
            at_sup = atp.tile([P, gsz * ROWS], F16, tag="at", name=f"at{g}")
            nc.sync.dma_start(
                at_sup[:].rearrange("p (q i) -> p q i", i=ROWS),
                adjt.rearrange("(q p) i -> p q i", p=P)[:, q0 : q0 + gsz, :],
            )
            xg = xtp2.tile([P, gsz * ROWS], F16, tag="xg", name=f"xg{g}")
            for qq in range(gsz):
                q = q0 + qq
                nc.vector.tensor_scalar_add(
                    xg[:, qq * ROWS : (qq + 1) * ROWS],
                    f1rep[:],
                    f2h_all[:, q : q + 1],
                )
            # t = tanh(s/2) in place (proven reader-rewriter pattern)
            nc.scalar.activation(xg[:], xg[:], AF.Tanh, bias=zero1[:])
            wg = wtp.tile([P, gsz * ROWS], F16, tag="wg", name=f"wg{g}")
            runs = []  # (start_qq, end_qq, is_quad)
            for qq in range(gsz):
                isq = (q0 + qq) in QSET
                if runs and runs[-1][2] == isq:
                    runs[-1][1] = qq + 1
                else:
                    runs.append([qq, qq + 1, isq])
            for r0, r1, isq in runs:
                sl = slice(r0 * ROWS, r1 * ROWS)
                if not isq:
                    # exact route: w = exp(0.5 t + 0.5)
                    nc.scalar.activation(
                        wg[:, sl], xg[:, sl], AF.Exp, bias=half1[:], scale=0.5
                    )
            return {"g": g, "gsz": gsz, "q0": q0, "at": at_sup,
                    "xg": xg, "wg": wg, "runs": runs}

        def emit_group_b(fr):
            """quad-route DVE ops + mask multiplies."""
            gsz, q0 = fr["gsz"], fr["q0"]
            at_sup, xg, wg, runs = fr["at"], fr["xg"], fr["wg"], fr["runs"]
            et = etp.tile([P, gsz * ROWS], F16, tag="et", name=f"et{fr['g']}")
            for r0, r1, isq in runs:
                sl = slice(r0 * ROWS, r1 * ROWS)
                if isq:
                    # quad: u = RHO*t + 2*D*RHO (ts 4x), y = u*t (tt 2x),
                    # et = (y + CPQ)*A (stt, single writer per range)
                    nc.vector.tensor_scalar(
                        wg[:, sl], xg[:, sl],
                        float(RHO), float(2 * D * RHO), ALU.mult, ALU.add,
                    )
                    yq = wtp.tile(
                        [P, (r1 - r0) * ROWS], F16, tag="yq", name=f"yq{fr['g']}"
                    )
                    nc.vector.tensor_mul(yq[:], wg[:, sl], xg[:, sl])
                    yq2 = wtp.tile(
                        [P, (r1 - r0) * ROWS], F16, tag="yq2",
                        name=f"yq2{fr['g']}"
                    )
                    nc.vector.tensor_scalar_add(yq2[:], yq[:], CPQ)
                    nc.vector.tensor_mul(et[:, sl], at_sup[:, sl], yq2[:])
                else:
                    # exact route mask: et = A * w
                    nc.vector.tensor_mul(
                        et[:, sl], at_sup[:, sl], wg[:, sl]
                    )
            return {"g": fr["g"], "gsz": gsz, "q0": q0, "et": et}

        def emit_group_back(fr, po_all, mid_a=None, mid_b=None):
            gsz, q0, et = fr["gsz"], fr["q0"], fr["et"]
            for qq in range(gsz):
                if qq == min(1, gsz - 1) and mid_a is not None:
                    mid_a()
                if qq == min(3, gsz - 1) and mid_b is not None:
                    mid_b()
                q = q0 + qq
                rhs = h16_all[:, q * HCOLS : (q + 1) * HCOLS]
                for it in range(NI):
                    nc.tensor.matmul(
                        po_all[:, it * BANK : it * BANK + HCOLS],
                        et[:, qq * ROWS + it * P : qq * ROWS + (it + 1) * P],
                        rhs,
                        start=(q == 0),
                        stop=(q == NT - 1),
                    )

        # ---- h-projection ----
        next_group = 0
        with tc.tile_pool(name="php", bufs=1, space="PSUM") as php:
            ph_all = php.tile([P, NI * BANK], F32, tag="ph")
            for b in range(NT // 4):
                for half in range(2):
                    nt0 = 4 * b + 2 * half
                    w0 = (nt0 % NI) * BANK
                    w1 = ((nt0 + 1) % NI) * BANK
                    for k in range(nkc):
                        nc.tensor.matmul(
                            ph_all[:, w0 : w0 + WCOLS],
                            xts[k][:, nt0 * P : (nt0 + 1) * P],
                            wes[k][:],
                            start=(k == 0),
                            stop=(k == nkc - 1),
                        )
                        nc.tensor.matmul(
                            ph_all[:, w1 : w1 + WCOLS],
                            xts[k][:, (nt0 + 1) * P : (nt0 + 2) * P],
                            wes[k][:],
                            start=(k == 0),
                            stop=(k == nkc - 1),
                        )
                bt = 4 * b
                wlo = (bt % NI) * BANK
                src = ph_all[:, wlo : wlo + 4 * BANK].rearrange(
                    "p (b w) -> p b w", b=4
                )
                dst_h = h16_all[:, bt * HCOLS : (bt + 4) * HCOLS].rearrange(
                    "p (b w) -> p b w", b=4
                )
                hc = C_OUT if b_zero else HCOLS
                nc.vector.tensor_copy(dst_h[:, :, 0:hc], src[:, :, 0:hc])
                if bt >= 8:
                    nc.vector.tensor_copy(
                        f2h_all[:, bt : bt + 4],
                        src[:, :, C_OUT + 2 : C_OUT + 3],
                    )
                while (
                    next_group < len(GROUPS)
                    and group_q0[next_group] + GROUPS[next_group] <= 4 * (b + 1)
                    and len(fa_list) + len(fb_list) < 2
                ):
                    fa_list.append(emit_group_a(next_group))
                    next_group += 1
                if len(fa_list) >= 2 and not fb_list:
                    fb_list.append(emit_group_b(fa_list.pop(0)))

        # ---- aggregate accumulators ----
        pop = ctx.enter_context(tc.tile_pool(name="po", bufs=1, space="PSUM"))
        po_all = pop.tile([P, NI * BANK], F32, tag="poall")

        # steady pipeline: back(g) mid-emits stage-A(g+2) then stage-B(g+1)
        def advance_a():
            nonlocal next_group
            if next_group < len(GROUPS):
                fa_list.append(emit_group_a(next_group))
                next_group += 1

        def advance_b():
            if fa_list:
                fb_list.append(emit_group_b(fa_list.pop(0)))

        while not fb_list:
            if not fa_list:
                advance_a()
            advance_b()
        while fb_list:
            fr = fb_list.pop(0)
            emit_group_back(fr, po_all, mid_a=advance_a, mid_b=advance_b)

        # ---- epilogue ----
        ns = obp.tile([P, NI * HCOLS], F32, tag="ns")
        ns3 = ns[:].rearrange("p (t c) -> p t c", c=HCOLS)
        dm = obp.tile([P, NI], F32, tag="dm")
        for it in range(NI):
            if it % 2 == 0:
                nc.vector.tensor_copy(
                    ns3[:, it, :], po_all[:, it * BANK : it * BANK + HCOLS]
                )
            else:
                nc.scalar.copy(
                    ns3[:, it, :], po_all[:, it * BANK : it * BANK + HCOLS]
                )
            nc.vector.tensor_scalar_max(
                dm[:, it : it + 1], ns3[:, it, C_OUT : C_OUT + 1], TINY
            )
        rc = obp.tile([P, NI], F32, tag="rc")
        nc.vector.reciprocal(rc[:], dm[:])
        ob_all = obp.tile([P, NI * C_OUT], F32, tag="oball")
        for it in range(NI):
            if it % 2 == 0:
                nc.vector.tensor_scalar_mul(
                    ob_all[:, it * C_OUT : (it + 1) * C_OUT],
                    ns3[:, it, 0:C_OUT],
                    rc[:, it : it + 1],
                )
            else:
                nc.scalar.mul(
                    ob_all[:, it * C_OUT : (it + 1) * C_OUT],
                    ns3[:, it, 0:C_OUT],
                    rc[:, it : it + 1],
                )
        nc.sync.dma_start(
            out.rearrange("(t p) c -> p t c", p=P),
            ob_all[:].rearrange("p (t c) -> p t c", c=C_OUT),
        )


def _prep_inputs(node_feats, adj_matrix, W, b, v0, v1):
    X = np.ascontiguousarray(node_feats, dtype=np.float32)
    W = np.asarray(W, dtype=np.float32)
    b = np.asarray(b, dtype=np.float32)
    v0 = np.asarray(v0, dtype=np.float32)
    v1 = np.asarray(v1, dtype=np.float32)

    w0h = (0.5 * (W.astype(np.float64) @ v0.astype(np.float64))).astype(np.float32)
    w1h = (0.5 * (W.astype(np.float64) @ v1.astype(np.float64))).astype(np.float32)
    c0h = np.float32(0.5 * float(b.astype(np.float64) @ v0.astype(np.float64)))
    c1h = np.float32(0.5 * float(b.astype(np.float64) @ v1.astype(np.float64)))

    XT1 = np.empty((257, N), np.float32)
    XT1[:256] = X.T
    XT1[256] = 1.0

    WE = np.zeros((257, WCOLS), np.float32)
    WE[:256, :C_OUT] = W
    WE[256, :C_OUT] = b
    WE[256, C_OUT] = 1.0
    WE[:256, C_OUT + 1] = w0h
    WE[256, C_OUT + 1] = c0h
    WE[:256, C_OUT + 2] = w1h
    WE[256, C_OUT + 2] = c1h

    XT1h = XT1.astype(np.float16)
    WEh = WE.astype(np.float16)
    A16 = np.asarray(adj_matrix, dtype=np.float16)

    in_maps = []
    for c in range(NCORES):
        in_maps.append(
            {
                "xt1": XT1h,
                "xt1l": np.ascontiguousarray(XT1h[:, c * ROWS : (c + 1) * ROWS]),
                "wext": WEh,
                "adjt": np.ascontiguousarray(
                    A16[c * ROWS : (c + 1) * ROWS, :].T
                ),
            }
        )
    return in_maps


def _run(in_maps, trace=False, b_zero=True):
    key = f"nc_b{int(b_zero)}"
    if key not in _CACHE:
        _CACHE[key] = _build_nc(b_zero=b_zero)
    nc = _CACHE[key]
    res = run_bass_kernel_spmd(
        nc, in_maps, core_ids=list(range(NCORES)), trace=trace
    )
    full = np.concatenate(
        [res.results[c]["out"] for c in range(NCORES)], axis=0
    ).astype(np.float32)
    return full, res


def kernel(node_feats, adj_matrix, W, b, v0, v1):
    in_maps = _prep_inputs(node_feats, adj_matrix, W, b, v0, v1)
    trace = bool(int(os.environ.get("GAT_TRACE", "0")))
    b_zero = not bool(np.any(np.asarray(b)))
    full, _ = _run(in_maps, trace=trace, b_zero=b_zero)
    return full
